# revision 17
# baseline (speedup 1.0000x reference)
"""Trainium2 Bass kernel for nn_CannyEdge (16,3,512,512) -> (16,3,512,512).

v2: fp16 on-chip pipeline (validated offline: rel err ~1e-3 vs reference,
ZERO keep-mask flips on the fixed input, robust to +-4ulp perturbation).

Math (all 3 output channels identical; decisions in msq = gx^2+gy^2 domain):
  x = channel-sum(img)                      [host, f32 -> fp16]
  tv = 5-tap vertical gaussian (reflect)    [PE banded matmul, fp16 w/x]
  t  = 5-tap horizontal gaussian (reflect)  [DVE, factored into two 3-tap]
  gx = [1,2,1]^T x [1,0,-1], gy = [1,0,-1]^T x [1,2,1]  (zero pad) [PE]
  sqx,sqy = squares (row-masked via Act scale), msq = sqx+sqy
  pair-max NMS: pb_k = msq > max(msq@+d_k, msq@-d_k); sum over 16 samples
    on PE (sel matmul); keep_k = relu(sum-15)   [Act drain]
  class from signs of p = gx*gy and sqy-sqx; out = sqrt(msq*SC) * keep_class

Sharding: spatial row-strips (batch-global AND stays core-local). Core k owns
image rows [64k,64k+64) of ALL 16 samples; 2 bands x 32 rows; strips of
SIN=40 rows (32+4+4 halo) pack 3 samples per tile on partitions 0/40/80;
6 tiles per band. Vertical taps/shifts are PE banded matmuls (image-boundary
reflect/zero baked into per-core matrices); horizontal shifts are free-dim
APs; msq row-shifts (mu/md) are SBUF->SBUF partition-shifted DMAs.
"""

import os

import numpy as np

import concourse.bacc as bacc
import concourse.mybir as mybir
from concourse.mybir import AluOpType as Op
from concourse.tile import TileContext
from concourse.bass_utils import run_bass_kernel_spmd

F32 = mybir.dt.float32
F16 = mybir.dt.float16
AF = mybir.ActivationFunctionType

B, C, H, W = 16, 3, 512, 512
NCORES = 8
ROWS = H // NCORES          # 64 output rows per core
SH = 32                     # band output rows
HALO = 4
SIN = SH + 2 * HALO         # 40 strip rows
PACK = 3
NBANDS = 2
TPB = 6                     # tiles per band
NT = NBANDS * TPB
NP = PACK * SIN             # 120 partitions used
SEG = W + 4                 # 516: msqz/mu/md segment width (2+512+2)
BFREE = TPB * W             # 3072
NVAR = 4                    # matrix variants: (band h) x (full | single)

KSIZE, SIGMA = 5, 1.4
PAIRS = [(0, 1), (-1, 1), (-1, 0), (-1, -1)]  # E, NE, N, NW

MATNAMES = ("vs", "vsn", "dv", "dv2", "sel")


def _gauss():
    half = (KSIZE - 1) * 0.5
    x = np.linspace(-half, half, KSIZE, dtype=np.float32)
    pdf = np.exp(np.float32(-0.5) * (x / np.float32(SIGMA)) ** 2).astype(np.float32)
    g = (pdf / pdf.sum()).astype(np.float32)
    q = (g / g[2]).astype(np.float32)          # [q2, q1, 1, q1, q2]
    q2, q1 = float(q[0]), float(q[1])
    s = q1 / q2
    pr = 1.0 / q2 - 2.0
    disc = float(np.sqrt(np.float32(s * s - 4 * pr)))
    a_ = np.float32((s + disc) / 2)            # 3-tap factor taps
    b_ = np.float32((s - disc) / 2)
    a_ = np.float32(np.float16(a_))
    b_ = np.float32(np.float16(b_))
    k0c = np.float32(g[2])
    sc = np.float32(k0c ** 4)  # fold into sqrt (q taps used directly now)
    return q, float(a_), float(b_), float(sc)


def _slots(t):
    return [3 * t + j for j in range(PACK) if 3 * t + j < B]


def _band_lhsT(core, h, nslots, taps, offs, mode, out_lo, out_hi):
    """lhsT (K=NP, M=NP) fp16 for a vertical conv: out[m] = sum_k lhsT[k,m] x[k]."""
    M = np.zeros((NP, NP), np.float16)
    gr0 = ROWS * core + SH * h - HALO
    for j in range(nslots):
        for io in range(out_lo, out_hi + 1):
            if not (0 <= gr0 + io < H):
                continue
            for tap, d in zip(taps, offs):
                g = gr0 + io + d
                if 0 <= g < H:
                    isrc = io + d
                elif mode == "reflect":
                    g2 = -g if g < 0 else 2 * (H - 1) - g
                    isrc = g2 - gr0
                else:
                    continue
                M[SIN * j + isrc, SIN * j + io] += np.float16(tap)
    return M


def _sel3(nslots):
    """Sum bits rows 4..35 over slots, replicate to all slots."""
    M = np.zeros((NP, NP), np.float16)
    for j in range(nslots):
        for jp in range(PACK):
            for i in range(HALO, HALO + SH):
                M[SIN * j + i, SIN * jp + i] = 1.0
    return M


def _build_core_inputs(img, core):
    q, _, _, _ = _gauss()
    qh = q.astype(np.float16).astype(np.float32)

    xs = img.sum(axis=1, dtype=np.float32).astype(np.float16)  # (B,H,W)

    # 516-wide tiles: cols -2..513 with reflect halo cols at image edges
    xsr = np.pad(xs, ((0, 0), (0, 0), (2, 2)), mode="reflect")
    chin = np.zeros((NP, NT * SEG), np.float16)
    for h in range(NBANDS):
        gr0 = ROWS * core + SH * h - HALO
        for t in range(TPB):
            T = TPB * h + t
            for j, s in enumerate(_slots(t)):
                lo = max(0, gr0)
                hi = min(H, gr0 + SIN)
                chin[SIN * j + (lo - gr0):SIN * j + (hi - gr0),
                     T * SEG:(T + 1) * SEG] = xsr[s, lo:hi, :]

    mats = {n: np.zeros((NVAR, NP, NP), np.float16) for n in MATNAMES}
    for h in range(NBANDS):
        for single in (0, 1):
            v = 2 * h + single
            ns = 1 if single else PACK
            mats["vs"][v] = _band_lhsT(core, h, ns, [1.0, 2.0, 1.0],
                                       [-1, 0, 1], "zero", 3, SIN - 4)
            mats["vsn"][v] = -mats["vs"][v]
            mats["dv"][v] = _band_lhsT(core, h, ns, [1.0, -1.0],
                                       [-1, 1], "zero", 3, SIN - 4)
            mats["dv2"][v] = 2.0 * mats["dv"][v]
            mats["sel"][v] = _sel3(ns)

    def tr(a):  # (NVAR,NP,NP) -> (NP, NVAR*NP)
        return np.ascontiguousarray(a.transpose(1, 0, 2).reshape(NP, NVAR * NP))

    out = {n: tr(mats[n]) for n in MATNAMES}
    out["chin"] = chin

    # composite blur: 5 variants x 5 taps of q[d] * vb (h+v gaussian fused)
    vbq = np.zeros((NVAR * 5, NP, NP), np.float16)
    for h in range(NBANDS):
        for single in (0, 1):
            v = 2 * h + single
            ns = 1 if single else PACK
            vb = _band_lhsT(core, h, ns, list(qh),
                            [-2, -1, 0, 1, 2], "reflect", 2, SIN - 3)
            for d in range(5):
                vbq[5 * v + d] = (vb.astype(np.float32)
                                  * np.float32(qh[d])).astype(np.float16)
    out["vbq"] = np.ascontiguousarray(
        vbq.transpose(1, 0, 2).reshape(NP, NVAR * 5 * NP))

    # row mask per band: 1 where partition's global row is inside the image
    rmask = np.zeros((NP, NBANDS), np.float32)
    for h in range(NBANDS):
        gr0 = ROWS * core + SH * h - HALO
        for j in range(PACK):
            for i in range(SIN):
                if 0 <= gr0 + i < H:
                    rmask[SIN * j + i, h] = 1.0
    out["rmask"] = rmask
    return out


def _build_bass(reps=1, dup=()):
    _, a_, b_, sc = _gauss()

    nc = bacc.Bacc("TRN2", target_bir_lowering=False, debug=False,
                   num_devices=NCORES)

    class _Dup:
        def __init__(self, eng, on):
            self._eng, self._on = eng, on

        def __getattr__(self, name):
            fn = getattr(self._eng, name)
            if not callable(fn) or not self._on:
                return fn

            def wrap(*a, **k):
                fn(*a, **k)
                return fn(*a, **k)
            return wrap

    vec = _Dup(nc.vector, "dve" in dup)
    act = _Dup(nc.scalar, "act" in dup)
    pool = _Dup(nc.gpsimd, "pool" in dup)
    pe = _Dup(nc.tensor, "pe" in dup)

    chin = nc.dram_tensor("chin", [NP, NT * SEG], F16, kind="ExternalInput").ap()
    vbqD = nc.dram_tensor("vbq", [NP, NVAR * 5 * NP], F16,
                          kind="ExternalInput").ap()
    rmaskD = nc.dram_tensor("rmask", [NP, NBANDS], F32, kind="ExternalInput").ap()
    dmats = {n: nc.dram_tensor(n, [NP, NVAR * NP], F16,
                               kind="ExternalInput").ap() for n in MATNAMES}
    outp = nc.dram_tensor("outp", [NP, NBANDS * BFREE], F16,
                          kind="ExternalOutput").ap()

    with TileContext(nc) as tc:
        with (
            tc.tile_pool(name="const", bufs=1) as cpool,
            tc.tile_pool(name="chp", bufs=1) as chpool,
            tc.tile_pool(name="tile", bufs=2) as wpool,
            tc.tile_pool(name="band", bufs=1) as bpool,
            tc.tile_pool(name="ptt", bufs=1, space="PSUM") as pttp,
            tc.tile_pool(name="pgx", bufs=1, space="PSUM") as pgx,
            tc.tile_pool(name="pgy", bufs=1, space="PSUM") as pgy,
            tc.tile_pool(name="pv", bufs=1, space="PSUM") as pvpool,
        ):
            smats = {}
            for name in MATNAMES:
                mt = cpool.tile([NP, NVAR * NP], F16, tag=name, name=f"m_{name}")
                nc.sync.dma_start(out=mt[:], in_=dmats[name])
                smats[name] = mt
            rmask = cpool.tile([NP, NBANDS], F32, tag="rmask", name="rmask")
            nc.sync.dma_start(out=rmask[:], in_=rmaskD)
            bias15 = cpool.tile([NP, 1], F32, tag="bias15", name="bias15")
            vec.memset(bias15[:], -15.0)
            vbq = cpool.tile([NP, NVAR * 5 * NP], F16, tag="vbq", name="m_vbq")
            nc.sync.dma_start(out=vbq[:], in_=vbqD)
            ch_s = chpool.tile([NP, NT * SEG], F16, tag="ch", name="ch")
            nc.sync.dma_start(out=ch_s[:], in_=chin)

            def mat(name, h, t):
                v = 2 * h + (1 if len(_slots(t)) == 1 else 0)
                return smats[name][:, v * NP:(v + 1) * NP]

            def matq(h, t, d):
                v = 2 * h + (1 if len(_slots(t)) == 1 else 0)
                i = 5 * v + d
                return vbq[:, i * NP:(i + 1) * NP]

            for rep in range(reps):
                for h in range(NBANDS):
                    msqz = bpool.tile([NP, TPB * SEG], F16, tag="msqz",
                                      name=f"msqz{rep}_{h}")
                    mu = bpool.tile([NP, TPB * SEG], F16, tag="mu",
                                    name=f"mu{rep}_{h}")
                    md = bpool.tile([NP, TPB * SEG], F16, tag="md",
                                    name=f"md{rep}_{h}")
                    p_s = bpool.tile([NP, BFREE], F16, tag="p",
                                     name=f"p{rep}_{h}")
                    mz3 = msqz[:].rearrange("p (s c) -> p s c", c=SEG)
                    vec.memset(mz3[:, :, 0:2], 0.0)
                    vec.memset(mz3[:, :, 514:516], 0.0)
                    asel = bpool.tile([NP, BFREE], mybir.dt.uint8, tag="asel",
                                      name=f"asel{rep}_{h}")

                    for t in range(TPB):
                        T = TPB * h + t
                        sg = t * SEG

                        ptt = pttp.tile([NP, W], F32, tag="tt", name=f"tt{rep}_{h}{t}")
                        for d in range(5):
                            pe.matmul(ptt[:], matq(h, t, d),
                                      ch_s[:, T * SEG + d:T * SEG + d + W],
                                      start=(d == 0), stop=(d == 4))
                        tt = wpool.tile([NP, W], F16, tag="tts",
                                        name=f"tts{rep}_{h}{t}")
                        act.activation(tt[:], ptt[:], AF.Copy)

                        # sobel on PE (zero pad via matrices + col ranges)
                        gx = pgx.tile([NP, W], F32, tag="gx", name=f"gx{rep}_{h}{t}")
                        pe.matmul(gx[:, 1:512], mat("vs", h, t),
                                         tt[:, 0:511], start=True, stop=False)
                        pe.matmul(gx[:, 0:511], mat("vsn", h, t),
                                         tt[:, 1:512], start=False, stop=True)
                        gy = pgy.tile([NP, W], F32, tag="gy", name=f"gy{rep}_{h}{t}")
                        pe.matmul(gy[:, 1:512], mat("dv", h, t),
                                         tt[:, 0:511], start=True, stop=False)
                        pe.matmul(gy[:, 0:511], mat("dv", h, t),
                                         tt[:, 1:512], start=False, stop=False)
                        pe.matmul(gy[:], mat("dv2", h, t), tt[:],
                                         start=False, stop=True)

                        rm = rmask[:, h:h + 1]
                        sqx = wpool.tile([NP, W], F16, tag="sqx",
                                         name=f"sqx{rep}_{h}{t}")
                        act.activation(sqx[:], gx[:], AF.Square, scale=rm)
                        sqy = wpool.tile([NP, W], F16, tag="sqy",
                                         name=f"sqy{rep}_{h}{t}")
                        act.activation(sqy[:], gy[:], AF.Square, scale=rm)
                        gxs = wpool.tile([NP, W], F16, tag="gxs",
                                         name=f"gxs{rep}_{h}{t}")
                        act.activation(gxs[:], gx[:], AF.Copy)
                        gys = wpool.tile([NP, W], F16, tag="gys",
                                         name=f"gys{rep}_{h}{t}")
                        act.activation(gys[:], gy[:], AF.Copy)

                        vec.tensor_tensor(msqz[:, sg + 2:sg + 514],
                                                sqx[:], sqy[:], Op.add)
                        vec.tensor_tensor(p_s[:, t * W:(t + 1) * W],
                                                gxs[:], gys[:], Op.mult)
                        vec.tensor_tensor(asel[:, t * W:(t + 1) * W],
                                                sqy[:], sqx[:], Op.is_ge)

                    # shifted copies of msqz (SBUF->SBUF DMA), parity-tuned so
                    # every pair-max runs in the DVE 2x mode:
                    #  muO/mdO: row+-1 and col+1 (center at odd idx 3)
                    #  muE/mdE: row+-1 only     (center at even idx 2)
                    #  msh:     col+1 only      (center at odd... msh[i]=msqz[i-1])
                    muE = bpool.tile([NP, TPB * SEG], F16, tag="muE",
                                     name=f"muE{rep}_{h}")
                    mdE = bpool.tile([NP, TPB * SEG], F16, tag="mdE",
                                     name=f"mdE{rep}_{h}")
                    msh = bpool.tile([NP, TPB * SEG], F16, tag="msh",
                                     name=f"msh{rep}_{h}")
                    nc.sync.dma_start(out=msh[:, 1:TPB * SEG],
                                      in_=msqz[:, 0:TPB * SEG - 1])
                    for j in range(PACK):
                        pb_ = SIN * j
                        nc.sync.dma_start(
                            out=mu[pb_ + 1:pb_ + SIN, 1:TPB * SEG],
                            in_=msqz[pb_:pb_ + SIN - 1, 0:TPB * SEG - 1])
                        nc.sync.dma_start(
                            out=md[pb_:pb_ + SIN - 1, 1:TPB * SEG],
                            in_=msqz[pb_ + 1:pb_ + SIN, 0:TPB * SEG - 1])
                        nc.sync.dma_start(
                            out=muE[pb_ + 1:pb_ + SIN, 0:TPB * SEG],
                            in_=msqz[pb_:pb_ + SIN - 1, 0:TPB * SEG])
                        nc.sync.dma_start(
                            out=mdE[pb_:pb_ + SIN - 1, 0:TPB * SEG],
                            in_=msqz[pb_ + 1:pb_ + SIN, 0:TPB * SEG])

                    def seg3(plane, lo, wdt):
                        return plane[:].rearrange("p (s c) -> p s c", c=SEG)[
                            :, :, lo:lo + wdt]

                    # pair maxes; mu/md center at idx c+3, msqz center at c+2
                    mxe = bpool.tile([NP, BFREE], F16, tag="mxe", name=f"mxe{rep}{h}")
                    vec.tensor_tensor(
                        mxe[:].rearrange("p (s c) -> p s c", c=W),
                        seg3(msh, 2, W), seg3(msh, 4, W), Op.max)
                    mxn = bpool.tile([NP, BFREE], F16, tag="mxn", name=f"mxn{rep}{h}")
                    vec.tensor_tensor(
                        mxn[:].rearrange("p (s c) -> p s c", c=W),
                        seg3(muE, 2, W), seg3(mdE, 2, W), Op.max)
                    mxne = bpool.tile([NP, BFREE], F16, tag="mxne", name=f"mxne{rep}{h}")
                    vec.tensor_tensor(
                        mxne[:].rearrange("p (s c) -> p s c", c=W),
                        seg3(mu, 4, W), seg3(md, 2, W), Op.max)
                    mxnw = bpool.tile([NP, BFREE], F16, tag="mxnw", name=f"mxnw{rep}{h}")
                    vec.tensor_tensor(
                        mxnw[:].rearrange("p (s c) -> p s c", c=W),
                        seg3(mu, 2, W), seg3(md, 4, W), Op.max)

                    pbts = []
                    for k, mx in enumerate((mxe, mxne, mxn, mxnw)):
                        pbt = bpool.tile([NP, BFREE], F16, tag=f"pb{k}",
                                         name=f"pb{k}_{rep}{h}")
                        vec.tensor_tensor(
                            pbt[:].rearrange("p (s c) -> p s c", c=W),
                            mx[:].rearrange("p (s c) -> p s c", c=W),
                            seg3(msqz, 2, W), Op.is_lt)
                        pbts.append(pbt)

                    # batch-AND on PE: sum over slots+tiles, then relu(s-15)
                    xks = []
                    for k in range(4):
                        vps = pvpool.tile([NP, W], F32, tag=f"v{k}",
                                          name=f"vps{rep}_{h}{k}")
                        for t in range(TPB):
                            pe.matmul(vps[:], mat("sel", h, t),
                                             pbts[k][:, t * W:(t + 1) * W],
                                             start=(t == 0), stop=(t == TPB - 1))
                        xk = bpool.tile([NP, W], F16, tag=f"xk{k}",
                                        name=f"xk{k}_{rep}{h}")
                        act.activation(xk[:], vps[:], AF.Relu, bias=bias15[:, 0:1])
                        xks.append(xk)

                    import dataclasses as _dc

                    def rep6(apx):
                        return _dc.replace(apx, ap=[apx.ap[0], [0, TPB], apx.ap[1]])

                    def as3(apx):
                        return apx.rearrange("p (s w) -> p s w", w=W)

                    vsel = bpool.tile([NP, BFREE], mybir.dt.uint8, tag="vsel",
                                      name=f"vsel{rep}{h}")
                    vec.tensor_scalar(vsel[:], p_s[:], 0.0, None, Op.is_lt)
                    d01 = bpool.tile([NP, W], F16, tag="d01", name=f"d01{rep}{h}")
                    pool.tensor_tensor(d01[:], xks[1][:], xks[0][:],
                                            Op.subtract)
                    yp = bpool.tile([NP, BFREE], F16, tag="yp", name=f"yp{rep}{h}")
                    pool.tensor_tensor(as3(yp[:]), as3(asel[:]),
                                            rep6(d01[:]), Op.mult)
                    pool.tensor_tensor(as3(yp[:]), as3(yp[:]),
                                            rep6(xks[0][:]), Op.add)
                    # yn = x3 + asel*(x2-x3)  (exact on {0,1}; runs on Pool)
                    d23 = bpool.tile([NP, W], F16, tag="d23", name=f"d23{rep}{h}")
                    pool.tensor_tensor(d23[:], xks[2][:], xks[3][:],
                                            Op.subtract)
                    yn = bpool.tile([NP, BFREE], F16, tag="yn", name=f"yn{rep}{h}")
                    pool.tensor_tensor(as3(yn[:]), as3(asel[:]),
                                            rep6(d23[:]), Op.mult)
                    pool.tensor_tensor(as3(yn[:]), as3(yn[:]),
                                            rep6(xks[3][:]), Op.add)
                    vec.copy_predicated(yp[:], vsel[:], yn[:])

                    mag = bpool.tile([NP, BFREE], F16, tag="mag", name=f"mag{rep}{h}")
                    act.activation(mag[:].rearrange("p (s w) -> p s w", w=W),
                                         seg3(msqz, 2, W), AF.Sqrt, scale=sc)
                    out_s = bpool.tile([NP, BFREE], F16, tag="out", name=f"out{rep}{h}")
                    vec.tensor_tensor(out_s[:], mag[:], yp[:], Op.mult)

                    nc.sync.dma_start(out=outp[:, h * BFREE:(h + 1) * BFREE],
                                      in_=out_s[:])

    nc.compile()
    return nc


_NC_CACHE = None
_IN_MAPS_CACHE = {}


def kernel(img):
    global _NC_CACHE
    img = np.ascontiguousarray(np.asarray(img, dtype=np.float32))
    assert img.shape == (B, C, H, W)

    if _NC_CACHE is None:
        _NC_CACHE = _build_bass()
    nc = _NC_CACHE

    in_maps = [_build_core_inputs(img, core) for core in range(NCORES)]
    trace = bool(os.environ.get("CANNY_TRACE"))
    res = run_bass_kernel_spmd(nc, in_maps, core_ids=list(range(NCORES)),
                               trace=trace)
    if trace and res.exec_time_ns is not None:
        print(f"HW exec time: {res.exec_time_ns} ns")
        kernel.last_exec_ns = res.exec_time_ns

    out = np.zeros((B, C, H, W), np.float32)
    for core in range(NCORES):
        o = np.asarray(res.results[core]["outp"], np.float32)
        for h in range(NBANDS):
            r0b = ROWS * core + SH * h
            for t in range(TPB):
                for j, s in enumerate(_slots(t)):
                    blk = o[SIN * j + HALO:SIN * j + HALO + SH,
                            h * BFREE + t * W:h * BFREE + (t + 1) * W]
                    out[s, :, r0b:r0b + SH, :] = blk[None]
    return out


if __name__ == "__main__":
    img = np.load("/tmp/img.npy")
    out = kernel(img)
    exp = np.load("/tmp/expected.npy")
    d = np.abs(out - exp)
    print("absmax", d.max(), "n>1e-2", (d > 1e-2).sum(),
          "keepmis", ((out != 0) != (exp != 0)).sum())
